# Initial kernel scaffold
#
"""Trainium2 Bass kernel for masked attention.

Reference semantics (B=4, S=4096, D=64):
    qs = q / 8
    scores = qs @ k.T + log(mask)[:, None, :]     # mask keys
    w = softmax(scores, axis=-1)
    out = w @ v
    return out * mask[..., None] + qs * (1 - mask)[..., None]

Sharding: 8 cores = (batch b = c//2, query half h = c%2). Each core
computes attention for 2048 queries of one batch with the batch's full
K/V/mask. Queries are independent -> no collectives.

Per-core algorithm (keys-on-partitions layout):
    scoresT[k, q] = sum_d K[k,d] Q[q,d]        (PE, fp32r, 1 cyc/row)
    E = exp(scoresT * 0.125)                   (ACT, reads PSUM)
    outT[m, q]  = sum_k Vaug[k, m] E[k, q]     (PE, fp32r, accum in PSUM)
      where Vaug[k, 0:64] = mask_k * V[k, :],  Vaug[k, 64] = mask_k
    row 64 of outT is the softmax denominator (mask folded into Vaug, so
    exp needs no masking and no max-subtraction: scores ~ N(0,1)).
    Epilogue: transpose outT back to [q, 65], divide by col 64, blend
    with qs passthrough for masked queries.
"""

import numpy as np

import concourse.bacc as bacc
import concourse.bass as bass
import concourse.tile as tile
import concourse.mybir as mybir
from concourse.bass_utils import run_bass_kernel_spmd
from concourse.masks import make_identity

B, S, D = 4, 4096, 64
NCORES = 8
QSH = (B * S) // NCORES          # 2048 queries per core
NKB = S // 128                   # 32 key blocks
NQT = QSH // 128                 # 16 query tiles
QCH = 1024                       # query-chunk width (free dim of scoresT)
NQC = QSH // QCH                 # 2 query chunks

F32 = mybir.dt.float32
F32R = mybir.dt.float32r
BF16 = mybir.dt.bfloat16
MM_DT = BF16                     # dtype for QK/PV matmul operands
Exp = mybir.ActivationFunctionType.Exp
MUL = mybir.AluOpType.mult
ADD = mybir.AluOpType.add


def _emit(tc, nc, q_d, k_d, v_d, mk_d, mq_d, o_d, ablate=()):
    ctx_pools = []

    consts = tc.alloc_tile_pool(name="consts", bufs=1)
    sb = tc.alloc_tile_pool(name="sb", bufs=1)
    expp = tc.alloc_tile_pool(name="expp", bufs=3)
    otp = tc.alloc_tile_pool(name="otp", bufs=2)
    finp = tc.alloc_tile_pool(name="finp", bufs=4)
    ctx_pools += [consts, sb, expp, otp, finp]

    identity = consts.tile([128, 128], F32, name="identity")
    make_identity(nc, identity)
    # warm the ACT exp table before the pipeline needs it
    actwarm = consts.tile([1, 1], F32, name="actwarm")
    nc.scalar.activation(out=actwarm, in_=identity[0:1, 0:1], func=Exp)

    q3 = sb.tile([128, NQT, D], F32, name="q3")     # q natural, tile qt = rows qt*128+p
    k3 = sb.tile([128, NKB, D], F32, name="k3")
    v3 = sb.tile([128, NKB, D + 1], MM_DT, name="v3")  # V augmented with ones col
    v3f = (sb.tile([128, NKB, D], F32, name="v3f")
           if MM_DT != F32R else None)                 # fp32 staging for V
    mk = sb.tile([128, NKB], F32, name="mk_sb")
    mq = sb.tile([128, NQT], F32, name="mq_sb")
    s1 = sb.tile([128, NQT], F32, name="s1_sb")      # 0.125*(1-mq)
    qb3 = sb.tile([128, NQT, D], F32, name="qb3")    # qs*(1-mq) passthrough term
    qT = sb.tile([64, QSH], MM_DT, name="qT")
    kT = sb.tile([64, S], MM_DT, name="kT")

    # ---- input DMAs, one chunk per prep group, first-needed first ----
    qap = q_d.ap().rearrange("(p n) d -> p n d", p=128)
    kap = k_d.ap().rearrange("(p n) d -> p n d", p=128)
    vap = v_d.ap().rearrange("(p n) d -> p n d", p=128)

    def dma_q(g):
        nc.sync.dma_start(out=q3[:, 4 * g:4 * g + 4, :], in_=qap[:, 4 * g:4 * g + 4, :])

    def dma_k(g):
        nc.sync.dma_start(out=k3[:, 4 * g:4 * g + 4, :], in_=kap[:, 4 * g:4 * g + 4, :])

    n_dma = 2 if "d_dma" in ablate else 1
    for _r in range(n_dma):
        dma_k(0)
        dma_q(0)
        dma_q(1)
        for g in range(1, NKB // 4):
            dma_k(g)
    nc.sync.dma_start(out=mk, in_=mk_d.ap().rearrange("(p n) -> p n", p=128))
    nc.sync.dma_start(out=mq, in_=mq_d.ap().rearrange("(p n) -> p n", p=128))
    for h in range(4):
        if MM_DT == F32R:
            nc.sync.dma_start(out=v3[:, 8 * h:8 * h + 8, 0:D],
                              in_=vap[:, 8 * h:8 * h + 8, :].bitcast(F32R))
        else:
            nc.sync.dma_start(out=v3f[:, 8 * h:8 * h + 8, :],
                              in_=vap[:, 8 * h:8 * h + 8, :])
    for g in range(2, NQT // 4):
        dma_q(g)

    # s1 = 0.125 * (1 - mq) = mq * (-0.125) + 0.125
    nc.vector.tensor_scalar(s1, mq, -0.125, 0.125, MUL, ADD)

    # Vaug: column D is mask_k itself (ones * mask); scale V cols by mask
    nc.vector.tensor_copy(v3[:, :, D:D + 1],
                          mk.rearrange("p (n o) -> p n o", o=1))
    for kb in range(NKB):
        vin = v3[:, kb, 0:D] if MM_DT == F32R else v3f[:, kb, :]
        nc.vector.tensor_scalar_mul(v3[:, kb, 0:D], vin, mk[:, kb:kb + 1])

    # ---- prep: transpose K and Q via PE (out = in_.T @ I) ----
    # Order: Q groups for the first query chunk first, then K groups (the
    # main loop's first QK matmul needs qT[:, 0:1024] + kT group 0), then
    # the remaining Q groups.
    prep = tc.alloc_tile_pool(name="prep_ps", bufs=2, space="PSUM")

    n_prep = 2 if "d_prep" in ablate else 1

    def prep_q(g):
        tp = prep.tile([64, 512], F32, name=f"prep_q{g}", tag="prep")
        for _r in range(n_prep):
            for j in range(4):
                qt = 4 * g + j
                nc.tensor.transpose(tp[:, 128 * j:128 * (j + 1)], q3[:, qt, :], identity)
        for _r in range(n_prep):
            nc.vector.tensor_copy(qT[:, 512 * g:512 * (g + 1)], tp)

    def prep_k(g):
        tp = prep.tile([64, 512], F32, name=f"prep_k{g}", tag="prep")
        for _r in range(n_prep):
            for j in range(4):
                kb = 4 * g + j
                nc.tensor.transpose(tp[:, 128 * j:128 * (j + 1)], k3[:, kb, :], identity)
        for _r in range(n_prep):
            nc.vector.tensor_copy(kT[:, 512 * g:512 * (g + 1)], tp)

    if "prep" not in ablate:
        prep_k(0)
        prep_q(0)
        prep_q(1)
        for g in range(1, NKB // 4):
            prep_k(g)
        for g in range(2, NQT // 4):
            prep_q(g)
    prep.release()

    # passthrough term, computed off the critical path
    for qt in range(NQT):
        nc.vector.tensor_scalar_mul(qb3[:, qt, :], q3[:, qt, :], s1[:, qt:qt + 1])

    # ---- main loop ----
    ps_e = tc.alloc_tile_pool(name="ps_e", bufs=2, space="PSUM")
    ps_o = tc.alloc_tile_pool(name="ps_o", bufs=1, space="PSUM")
    ps_sc = tc.alloc_tile_pool(name="ps_sc", bufs=2, space="PSUM")
    ctx_pools += [ps_e, ps_o, ps_sc]

    for qc in range(NQC):
        oT_ps = ps_o.tile([D + 1, QCH], F32, name=f"oT_ps{qc}", tag="ot")
        for kb in range(NKB):
            sc = ps_sc.tile([128, QCH], F32, name=f"sc{qc}_{kb}", tag="sc")
            for _r in range(2 if "d_qk" in ablate else 1):
                for j in range(1 if "half_qk" in ablate else QCH // 512):
                    nc.tensor.matmul(
                        sc[:, 512 * j:512 * (j + 1)],
                        lhsT=kT[:, 128 * kb:128 * (kb + 1)],
                        rhs=qT[:, QCH * qc + 512 * j:QCH * qc + 512 * (j + 1)],
                        start=True, stop=True,
                    )
            ex = expp.tile([128, QCH], MM_DT, name=f"ex{qc}_{kb}", tag="ex")
            for _r in range(2 if "d_exp" in ablate else 1):
                nc.scalar.activation(out=ex, in_=sc, func=Exp, scale=0.125)
            for _r in range(2 if "d_pv" in ablate else 1):
                for j in range(1 if "half_pv" in ablate else QCH // 512):
                    nc.tensor.matmul(
                        oT_ps[:, 512 * j:512 * (j + 1)],
                        lhsT=v3[:, kb, :],
                        rhs=ex[:, 512 * j:512 * (j + 1)],
                        start=(kb == 0 and _r == 0), stop=(kb == NKB - 1),
                    )
        # epilogue for this query chunk
        oT_sb = otp.tile([D + 1, QCH], F32, name=f"oT_sb{qc}", tag="otsb")
        nc.vector.tensor_copy(oT_sb, oT_ps)
        fin3 = finp.tile([128, QCH // 128, D], F32, name=f"fin3_{qc}", tag="fin3",
                         bufs=2)
        for t in range(0 if "epi" in ablate else QCH // 128):
            qt = qc * (QCH // 128) + t
            tp = ps_e.tile([128, D + 1], F32, name=f"epi{qt}", tag="epi")
            nc.tensor.transpose(tp, oT_sb[:, 128 * t:128 * (t + 1)],
                                identity[0:D + 1, 0:D + 1])
            rec = finp.tile([128, 1], F32, name=f"rec{qt}", tag="rec")
            nc.vector.reciprocal(rec, tp[:, D:D + 1])
            recm = finp.tile([128, 1], F32, name=f"recm{qt}", tag="recm")
            nc.vector.tensor_scalar_mul(recm, rec, mq[:, qt:qt + 1])
            # fin = (pv * recm) + qb
            nc.vector.scalar_tensor_tensor(fin3[:, t, :], tp[:, 0:D], recm,
                                           qb3[:, qt, :], MUL, ADD)
        oap = o_d.ap().rearrange("(p n) d -> p n d", p=128)
        nc.sync.dma_start(
            out=oap[:, qc * (QCH // 128):(qc + 1) * (QCH // 128), :], in_=fin3)

    for p in reversed(ctx_pools):
        p.release()


_PROGS = {}


def _build(repeat=1, loop=None, ablate=()):
    key = (repeat, loop, tuple(ablate))
    if key in _PROGS:
        return _PROGS[key]
    nc = bacc.Bacc("TRN2", target_bir_lowering=False, debug=False)
    q_d = nc.dram_tensor("q_in", [QSH, D], F32, kind="ExternalInput")
    k_d = nc.dram_tensor("k_in", [S, D], F32, kind="ExternalInput")
    v_d = nc.dram_tensor("v_in", [S, D], F32, kind="ExternalInput")
    mk_d = nc.dram_tensor("mk_in", [S], F32, kind="ExternalInput")
    mq_d = nc.dram_tensor("mq_in", [QSH], F32, kind="ExternalInput")
    o_d = nc.dram_tensor("o_out", [QSH, D], F32, kind="ExternalOutput")
    with tile.TileContext(nc) as tc:
        if loop is not None:
            with tc.For_i(0, loop, 1):
                for _ in range(repeat):
                    _emit(tc, nc, q_d, k_d, v_d, mk_d, mq_d, o_d, ablate=ablate)
        else:
            for _ in range(repeat):
                _emit(tc, nc, q_d, k_d, v_d, mk_d, mq_d, o_d, ablate=ablate)
    nc.compile()
    _PROGS[key] = nc
    return nc


def make_in_maps(q, k, v, mask):
    q = np.ascontiguousarray(np.asarray(q, dtype=np.float32))
    k = np.ascontiguousarray(np.asarray(k, dtype=np.float32))
    v = np.ascontiguousarray(np.asarray(v, dtype=np.float32))
    mask = np.ascontiguousarray(np.asarray(mask, dtype=np.float32))
    in_maps = []
    for c in range(NCORES):
        b, h = c // 2, c % 2
        sl = slice(h * QSH, (h + 1) * QSH)
        in_maps.append({
            "q_in": np.ascontiguousarray(q[b, sl, :]),
            "k_in": np.ascontiguousarray(k[b]),
            "v_in": np.ascontiguousarray(v[b]),
            "mk_in": np.ascontiguousarray(mask[b]),
            "mq_in": np.ascontiguousarray(mask[b, sl]),
        })
    return in_maps


def gather(results):
    out = np.empty((B, S, D), np.float32)
    for c in range(NCORES):
        b, h = c // 2, c % 2
        out[b, h * QSH:(h + 1) * QSH, :] = results[c]["o_out"]
    return out


def kernel(q, k, v, mask, _spmd_kwargs=None):
    nc = _build()
    in_maps = make_in_maps(q, k, v, mask)
    res = run_bass_kernel_spmd(nc, in_maps, core_ids=list(range(NCORES)),
                               **(_spmd_kwargs or {}))
    out = gather(res.results)
    if _spmd_kwargs:
        kernel._last_results = res
    return out



# revision 1
# speedup vs baseline: 1.1534x; 1.1534x over previous
"""Trainium2 Bass kernel for masked attention.

Reference semantics (B=4, S=4096, D=64):
    qs = q / 8
    scores = qs @ k.T + log(mask)[:, None, :]     # mask keys
    w = softmax(scores, axis=-1)
    out = w @ v
    return out * mask[..., None] + qs * (1 - mask)[..., None]

Sharding: 8 cores = (batch b = c//2, query half h = c%2). Each core
computes attention for 2048 queries of one batch with the batch's full
K/V/mask. Queries are independent -> no collectives.

Per-core algorithm (keys-on-partitions layout):
    scoresT[k, q] = sum_d K[k,d] Q[q,d]        (PE, fp32r, 1 cyc/row)
    E = exp(scoresT * 0.125)                   (ACT, reads PSUM)
    outT[m, q]  = sum_k Vaug[k, m] E[k, q]     (PE, fp32r, accum in PSUM)
      where Vaug[k, 0:64] = mask_k * V[k, :],  Vaug[k, 64] = mask_k
    row 64 of outT is the softmax denominator (mask folded into Vaug, so
    exp needs no masking and no max-subtraction: scores ~ N(0,1)).
    Epilogue: transpose outT back to [q, 65], divide by col 64, blend
    with qs passthrough for masked queries.
"""

import numpy as np

import concourse.bacc as bacc
import concourse.bass as bass
import concourse.tile as tile
import concourse.mybir as mybir
from concourse.bass_utils import run_bass_kernel_spmd
from concourse.masks import make_identity

B, S, D = 4, 4096, 64
NCORES = 8
QSH = (B * S) // NCORES          # 2048 queries per core
NKB = S // 128                   # 32 key blocks
NQT = QSH // 128                 # 16 query tiles
QCH = 1024                       # query-chunk width (free dim of scoresT)
NQC = QSH // QCH                 # 2 query chunks

F32 = mybir.dt.float32
F32R = mybir.dt.float32r
BF16 = mybir.dt.bfloat16
MM_DT = BF16                     # dtype for QK/PV matmul operands
Exp = mybir.ActivationFunctionType.Exp
MUL = mybir.AluOpType.mult
ADD = mybir.AluOpType.add


def _emit(tc, nc, q_d, k_d, v_d, mk_d, mq_d, o_d, ablate=()):
    ctx_pools = []

    consts = tc.alloc_tile_pool(name="consts", bufs=1)
    sb = tc.alloc_tile_pool(name="sb", bufs=1)
    expp = tc.alloc_tile_pool(name="expp", bufs=3)
    otp = tc.alloc_tile_pool(name="otp", bufs=2)
    finp = tc.alloc_tile_pool(name="finp", bufs=4)
    ctx_pools += [consts, sb, expp, otp, finp]

    identity = consts.tile([128, 128], F32, name="identity")
    make_identity(nc, identity)
    # warm the ACT exp table before the pipeline needs it
    actwarm = consts.tile([1, 1], F32, name="actwarm")
    nc.scalar.activation(out=actwarm, in_=identity[0:1, 0:1], func=Exp)

    q3 = sb.tile([128, NQT, D], F32, name="q3")     # q natural, tile qt = rows qt*128+p
    k3 = sb.tile([128, NKB, D], F32, name="k3")
    v3 = sb.tile([128, NKB, D + 1], MM_DT, name="v3")  # V augmented with ones col
    v3f = (sb.tile([128, NKB, D], F32, name="v3f")
           if MM_DT != F32R else None)                 # fp32 staging for V
    mk = sb.tile([128, NKB], F32, name="mk_sb")
    mq = sb.tile([128, NQT], F32, name="mq_sb")
    s1 = sb.tile([128, NQT], F32, name="s1_sb")      # 0.125*(1-mq)
    qb3 = sb.tile([128, NQT, D], F32, name="qb3")    # qs*(1-mq) passthrough term
    qT = sb.tile([64, QSH], MM_DT, name="qT")
    kT = sb.tile([64, S], MM_DT, name="kT")

    # ---- input DMAs, one chunk per prep group, first-needed first ----
    qap = q_d.ap().rearrange("(p n) d -> p n d", p=128)
    kap = k_d.ap().rearrange("(p n) d -> p n d", p=128)
    vap = v_d.ap().rearrange("(p n) d -> p n d", p=128)

    def dma_q(g):
        nc.sync.dma_start(out=q3[:, 4 * g:4 * g + 4, :], in_=qap[:, 4 * g:4 * g + 4, :])

    def dma_k(g):
        nc.sync.dma_start(out=k3[:, 4 * g:4 * g + 4, :], in_=kap[:, 4 * g:4 * g + 4, :])

    n_dma = 2 if "d_dma" in ablate else 1
    for _r in range(n_dma):
        dma_k(0)
        dma_q(0)
        dma_q(1)
        for g in range(1, NKB // 4):
            dma_k(g)
    nc.sync.dma_start(out=mk, in_=mk_d.ap().rearrange("(p n) -> p n", p=128))
    nc.sync.dma_start(out=mq, in_=mq_d.ap().rearrange("(p n) -> p n", p=128))
    for h in range(4):
        if MM_DT == F32R:
            nc.sync.dma_start(out=v3[:, 8 * h:8 * h + 8, 0:D],
                              in_=vap[:, 8 * h:8 * h + 8, :].bitcast(F32R))
        else:
            nc.sync.dma_start(out=v3f[:, 8 * h:8 * h + 8, :],
                              in_=vap[:, 8 * h:8 * h + 8, :])
    for g in range(2, NQT // 4):
        dma_q(g)

    # s1 = 0.125 * (1 - mq) = mq * (-0.125) + 0.125
    nc.vector.tensor_scalar(s1, mq, -0.125, 0.125, MUL, ADD)

    # Vaug: column D is mask_k itself (ones * mask); scale V cols by mask
    nc.vector.tensor_copy(v3[:, :, D:D + 1],
                          mk.rearrange("p (n o) -> p n o", o=1))
    for kb in range(NKB):
        vin = v3[:, kb, 0:D] if MM_DT == F32R else v3f[:, kb, :]
        nc.vector.tensor_scalar_mul(v3[:, kb, 0:D], vin, mk[:, kb:kb + 1])

    # ---- prep: transpose K and Q via PE (out = in_.T @ I) ----
    # Order: Q groups for the first query chunk first, then K groups (the
    # main loop's first QK matmul needs qT[:, 0:1024] + kT group 0), then
    # the remaining Q groups.
    prep = tc.alloc_tile_pool(name="prep_ps", bufs=2, space="PSUM")

    n_prep = 2 if "d_prep" in ablate else 1

    def prep_q(g):
        tp = prep.tile([64, 512], F32, name=f"prep_q{g}", tag="prep")
        for _r in range(n_prep):
            for j in range(4):
                qt = 4 * g + j
                nc.tensor.transpose(tp[:, 128 * j:128 * (j + 1)], q3[:, qt, :], identity)
        for _r in range(n_prep):
            nc.vector.tensor_copy(qT[:, 512 * g:512 * (g + 1)], tp)

    def prep_k(g):
        tp = prep.tile([64, 512], F32, name=f"prep_k{g}", tag="prep")
        for _r in range(n_prep):
            for j in range(4):
                kb = 4 * g + j
                nc.tensor.transpose(tp[:, 128 * j:128 * (j + 1)], k3[:, kb, :], identity)
        for _r in range(n_prep):
            nc.vector.tensor_copy(kT[:, 512 * g:512 * (g + 1)], tp)

    if "prep" not in ablate:
        prep_k(0)
        prep_q(0)
        prep_q(1)
        for g in range(1, NKB // 4):
            prep_k(g)
        for g in range(2, NQT // 4):
            prep_q(g)
    prep.release()

    # passthrough term, computed off the critical path
    for qt in range(NQT):
        nc.vector.tensor_scalar_mul(qb3[:, qt, :], q3[:, qt, :], s1[:, qt:qt + 1])

    # ---- main loop ----
    ps_e = tc.alloc_tile_pool(name="ps_e", bufs=2, space="PSUM")
    ps_o = tc.alloc_tile_pool(name="ps_o", bufs=1, space="PSUM")
    ps_sc = tc.alloc_tile_pool(name="ps_sc", bufs=2, space="PSUM")
    ctx_pools += [ps_e, ps_o, ps_sc]

    for qc in range(NQC):
        oT_ps = ps_o.tile([D + 1, QCH], F32, name=f"oT_ps{qc}", tag="ot")
        for kb in range(NKB):
            sc = ps_sc.tile([128, QCH], F32, name=f"sc{qc}_{kb}", tag="sc")
            for _r in range(2 if "d_qk" in ablate else 1):
                for j in range(1 if "half_qk" in ablate else QCH // 512):
                    nc.tensor.matmul(
                        sc[:, 512 * j:512 * (j + 1)],
                        lhsT=kT[:, 128 * kb:128 * (kb + 1)],
                        rhs=qT[:, QCH * qc + 512 * j:QCH * qc + 512 * (j + 1)],
                        start=True, stop=True,
                    )
            ex = expp.tile([128, QCH], MM_DT, name=f"ex{qc}_{kb}", tag="ex")
            for _r in range(2 if "d_exp" in ablate else 1):
                nc.scalar.activation(out=ex, in_=sc, func=Exp, scale=0.125)
            for _r in range(2 if "d_pv" in ablate else 1):
                for j in range(1 if "half_pv" in ablate else QCH // 512):
                    nc.tensor.matmul(
                        oT_ps[:, 512 * j:512 * (j + 1)],
                        lhsT=v3[:, kb, :],
                        rhs=ex[:, 512 * j:512 * (j + 1)],
                        start=(kb == 0 and _r == 0), stop=(kb == NKB - 1),
                    )
        # epilogue for this query chunk
        oT_sb = otp.tile([D + 1, QCH], F32, name=f"oT_sb{qc}", tag="otsb")
        nc.vector.tensor_copy(oT_sb, oT_ps)
        fin3 = finp.tile([128, QCH // 128, D], F32, name=f"fin3_{qc}", tag="fin3",
                         bufs=2)
        for t in range(0 if "epi" in ablate else QCH // 128):
            qt = qc * (QCH // 128) + t
            tp = ps_e.tile([128, D + 1], F32, name=f"epi{qt}", tag="epi")
            nc.tensor.transpose(tp, oT_sb[:, 128 * t:128 * (t + 1)],
                                identity[0:D + 1, 0:D + 1])
            rec = finp.tile([128, 1], F32, name=f"rec{qt}", tag="rec")
            nc.vector.reciprocal(rec, tp[:, D:D + 1])
            recm = finp.tile([128, 1], F32, name=f"recm{qt}", tag="recm")
            nc.vector.tensor_scalar_mul(recm, rec, mq[:, qt:qt + 1])
            # fin = (pv * recm) + qb
            nc.vector.scalar_tensor_tensor(fin3[:, t, :], tp[:, 0:D], recm,
                                           qb3[:, qt, :], MUL, ADD)
        oap = o_d.ap().rearrange("(p n) d -> p n d", p=128)
        nc.sync.dma_start(
            out=oap[:, qc * (QCH // 128):(qc + 1) * (QCH // 128), :], in_=fin3)

    for p in reversed(ctx_pools):
        p.release()


_PROGS = {}


def _build(repeat=1, loop=None, ablate=()):
    key = (repeat, loop, tuple(ablate))
    if key in _PROGS:
        return _PROGS[key]
    nc = bacc.Bacc("TRN2", target_bir_lowering=False, debug=False)
    q_d = nc.dram_tensor("q_in", [QSH, D], F32, kind="ExternalInput")
    k_d = nc.dram_tensor("k_in", [S, D], F32, kind="ExternalInput")
    v_d = nc.dram_tensor("v_in", [S, D], F32, kind="ExternalInput")
    mk_d = nc.dram_tensor("mk_in", [S], F32, kind="ExternalInput")
    mq_d = nc.dram_tensor("mq_in", [QSH], F32, kind="ExternalInput")
    o_d = nc.dram_tensor("o_out", [QSH, D], F32, kind="ExternalOutput")
    with tile.TileContext(nc) as tc:
        if loop is not None:
            with tc.For_i(0, loop, 1):
                for _ in range(repeat):
                    _emit(tc, nc, q_d, k_d, v_d, mk_d, mq_d, o_d, ablate=ablate)
        else:
            for _ in range(repeat):
                _emit(tc, nc, q_d, k_d, v_d, mk_d, mq_d, o_d, ablate=ablate)
    nc.compile()
    _PROGS[key] = nc
    return nc


def make_in_maps(q, k, v, mask):
    q = np.ascontiguousarray(np.asarray(q, dtype=np.float32))
    k = np.ascontiguousarray(np.asarray(k, dtype=np.float32))
    v = np.ascontiguousarray(np.asarray(v, dtype=np.float32))
    mask = np.ascontiguousarray(np.asarray(mask, dtype=np.float32))
    in_maps = []
    for c in range(NCORES):
        b, h = c // 2, c % 2
        sl = slice(h * QSH, (h + 1) * QSH)
        in_maps.append({
            "q_in": np.ascontiguousarray(q[b, sl, :]),
            "k_in": np.ascontiguousarray(k[b]),
            "v_in": np.ascontiguousarray(v[b]),
            "mk_in": np.ascontiguousarray(mask[b]),
            "mq_in": np.ascontiguousarray(mask[b, sl]),
        })
    return in_maps


def gather(results):
    out = np.empty((B, S, D), np.float32)
    for c in range(NCORES):
        b, h = c // 2, c % 2
        out[b, h * QSH:(h + 1) * QSH, :] = results[c]["o_out"]
    return out


def kernel(q, k, v, mask, _spmd_kwargs=None):
    nc = _build()
    in_maps = make_in_maps(q, k, v, mask)
    res = run_bass_kernel_spmd(nc, in_maps, core_ids=list(range(NCORES)),
                               **(_spmd_kwargs or {}))
    out = gather(res.results)
    if _spmd_kwargs:
        kernel._last_results = res
    return out

